# revision 5
# baseline (speedup 1.0000x reference)
"""Trainium2 Bass kernel for nn_BackBone (LSTM backbone + fc + outer-product head).

Data-parallel over batch across 8 NeuronCores. Per core (b_loc=1024), v5:
  - hybrid-precision LSTM: steps t < TC=14 run the input projection AND the
    recurrence as fp8-e4m3 DoubleRow matmuls (2 contraction rows/cell/cycle);
    steps t >= TC run fp16 like v4. Forget-gate decay kills the early fp8
    quantization error (measured 5e-3 final rel err vs 2e-2 budget).
  - fp8 step PE stream/chain/gate: DR(x[0:256]) + DR([h8 ; x[256:340]+bias])
    -- the recurrence h and the x-tail share one 256-contraction DR matmul.
    h8(t-1) and xtail(t) pair up in one [128,2,512] AP because the host
    stores xtail plane-shifted by one step in the shared hx8 tile.
  - PSUM per chain: Pifo [128,3,512] (one merged sigmoid(i,f,o) ACT read)
    + Pg (tanh). ACT per chain per step: sigmoid 1536 + tanh 512 + tanh(c)
    512 cols -> ~6.1us/step both chains = the pacing engine.
  - y2 head einsum: DVE broadcast tensor_tensor [128,nl,3,128] jobs (1x mode
    but big ops beat per-(l,m) tensor_scalar) + GpSimd jobs, zero ACT during
    the recurrence; stores on gpsimd SWDGE.
  - y1 tail: dma_start_transpose h on sync+tensor queues, einsum split
    DVE/GpSimd/ACT (all idle post-LSTM), stores on sync+tensor.
"""
import numpy as np
import ml_dtypes

import concourse.bacc as bacc
import concourse.mybir as mybir
import concourse.tile as tile
from concourse import bass_utils

F32 = mybir.dt.float32
F16 = mybir.dt.float16
F8 = mybir.dt.float8e4
AF = mybir.ActivationFunctionType
DR = mybir.MatmulPerfMode.DoubleRow

T = 20
TC = 14                  # steps 0..TC-1 in fp8 DoubleRow
D = 340
DP = D + 1               # +1 constant feature row carrying the gate bias
H = 128
E = 32
L = 10
M3 = 3
DCH = [(0, 128), (128, 256), (256, DP)]   # fp16 contraction chunks
N_CORES = 8
E4NP = ml_dtypes.float8_e4m3


def build_program(b_loc: int):
    assert b_loc % 256 == 0
    NJ = b_loc // 128
    CW = b_loc // 2               # chain width (<= 512)
    assert CW <= 512
    NCB = 2
    T16 = T - TC                  # fp16 steps

    nc = bacc.Bacc("TRN2", target_bir_lowering=False, debug=False)
    # fp8 proj rows 0:256 as 2 k-slots
    x8_d = nc.dram_tensor("x8", (128, 2, TC, b_loc), F8,
                          kind="ExternalInput").ap()
    # fp8 x-tail (rows 256:340 + bias row + zero pad), plane-shifted by -1:
    # plane p holds xtail[t=p+1]; plane TC-1 holds xtail[t=0]
    x8t_d = nc.dram_tensor("x8t", (128, TC, b_loc), F8,
                           kind="ExternalInput").ap()
    # fp16 x for steps TC..T-1 (DP rows incl bias-ones row), chunked
    x16_d = [nc.dram_tensor(f"x16_{k}", (c1 - c0, T16, b_loc), F16,
                            kind="ExternalInput").ap()
             for k, (c0, c1) in enumerate(DCH)]
    wp8_d = nc.dram_tensor("wp8", (128, 2, 4 * H), F8,
                           kind="ExternalInput").ap()
    wc8_d = nc.dram_tensor("wc8", (128, 2, 4 * H), F8,
                           kind="ExternalInput").ap()
    w16_d = [nc.dram_tensor(f"w16_{k}", (c1 - c0, 4 * H), F16,
                            kind="ExternalInput").ap()
             for k, (c0, c1) in enumerate(DCH)]
    whh_d = nc.dram_tensor("w_hh_t", (H, 4 * H), F16, kind="ExternalInput").ap()
    cnt_d = nc.dram_tensor("cn_t", (E, b_loc), F16, kind="ExternalInput").ap()
    fcw_d = nc.dram_tensor("fc_w_t", (E, H), F16, kind="ExternalInput").ap()
    fcb_d = nc.dram_tensor("fc_b_row", (1, H), F16, kind="ExternalInput").ap()
    ones_d = nc.dram_tensor("ones_row", (1, 128), F16, kind="ExternalInput").ap()
    pref_d = nc.dram_tensor("pref_g", (128, NJ, L, M3), F16,
                            kind="ExternalInput").ap()
    pref32_d = nc.dram_tensor("pref_g32", (128, NJ, L, M3), F32,
                              kind="ExternalInput").ap()
    oy1 = nc.dram_tensor("out_y1", (b_loc, L, M3, 128), F16,
                         kind="ExternalOutput").ap()
    oy2 = nc.dram_tensor("out_y2", (b_loc, L, M3, 128), F16,
                         kind="ExternalOutput").ap()

    TGR8 = [(0, 1), (1, 2), (2, 4), (4, 7), (7, 10), (10, TC)]
    TGR16 = [(0, 3), (3, T16)]

    with tile.TileContext(nc) as tc:
        with tc.tile_pool(name="wpool", bufs=1) as wpool, \
             tc.tile_pool(name="main", bufs=1) as pool, \
             tc.tile_pool(name="psum", bufs=1, space="PSUM") as pspool:

            # ---- hx8: slot0 = h8(t) planes, slot1 = xtail shifted ----
            hx8 = pool.tile([128, 2, TC, b_loc], F8, name="hx8", tag="hx8")
            # single DMA for the whole xtail so later h8 writes only ever
            # WAW-wait on this one early load (scalar queue: ACT idle now)
            nc.scalar.dma_start(hx8[:, 1, :, :], x8t_d)

            # ---- weights / constants ----
            wp8_t = wpool.tile([128, 2, 4 * H], F8, name="wp8_t")
            nc.sync.dma_start(wp8_t[:], wp8_d)
            wc8_t = wpool.tile([128, 2, 4 * H], F8, name="wc8_t")
            nc.sync.dma_start(wc8_t[:], wc8_d)
            w16_t = []
            for k, (c0, c1) in enumerate(DCH):
                wt_ = wpool.tile([c1 - c0, 4 * H], F16, name=f"w16{k}")
                nc.sync.dma_start(wt_[:], w16_d[k])
                w16_t.append(wt_)
            whh_t = wpool.tile([H, 4 * H], F16, name="whh_t")
            nc.sync.dma_start(whh_t[:], whh_d)
            cnt_t = wpool.tile([E, b_loc], F16, name="cnt_t")
            nc.gpsimd.dma_start(cnt_t[:], cnt_d)
            fcw_t = wpool.tile([E, H], F16, name="fcw_t")
            nc.gpsimd.dma_start(fcw_t[:], fcw_d)
            fcb_t = wpool.tile([1, H], F16, name="fcb_t")
            nc.gpsimd.dma_start(fcb_t[:], fcb_d)
            ones_t = wpool.tile([1, 128], F16, name="ones_t")
            nc.gpsimd.dma_start(ones_t[:], ones_d)
            pf_t = wpool.tile([128, NJ, L, M3], F16, name="pf_t")
            nc.gpsimd.dma_start(pf_t[:], pref_d)
            pf32_t = wpool.tile([128, NJ, L, M3], F32, name="pf32_t")
            nc.gpsimd.dma_start(pf32_t[:], pref32_d)

            # ---- x tiles, loaded in t-groups ----
            x8_t = pool.tile([128, 2, TC, b_loc], F8, name="x8_t", tag="x8")
            for (t0, t1) in TGR8:
                nc.sync.dma_start(x8_t[:, :, t0:t1, :], x8_d[:, :, t0:t1, :])
            x16_t = []
            for k, (c0, c1) in enumerate(DCH):
                x16_t.append(pool.tile([c1 - c0, T16, b_loc], F16,
                                       name=f"x16t{k}", tag=f"x16{k}"))
            for (t0, t1) in TGR16:
                for k in range(3):
                    nc.sync.dma_start(x16_t[k][:, t0:t1, :],
                                      x16_d[k][:, t0:t1, :])

            # ---- PSUM: per chain Pifo (3 banks) + Pg (1 bank) ----
            Pifo, Pg = [], []
            for cb in range(NCB):
                Pifo.append(pspool.tile([128, 3, 512], F32, name=f"pifo{cb}",
                                        tag=f"pifo{cb}"))
                Pg.append(pspool.tile([128, 512], F32, name=f"pg{cb}",
                                      tag=f"pg{cb}"))

            def emit_einsum_bcast(j, y_half, odram, l0, nl, engine,
                                  store_eng):
                ol = pool.tile([128, nl, M3, 128], F16, name="ol",
                               tag="outl", bufs=10)
                y_b = y_half[:, None, None, :].broadcast_to(
                    [128, nl, M3, 128])
                p_b = pf_t[:, j, l0:l0 + nl, :, None].broadcast_to(
                    [128, nl, M3, 128])
                engine.tensor_mul(ol[:], y_b, p_b)
                store_eng.dma_start(
                    odram[j * 128:(j + 1) * 128, l0:l0 + nl, :, :], ol[:])

            def emit_einsum_act(j, y_half, odram, l0, nl, store_eng):
                ol = pool.tile([128, nl, M3, 128], F16, name="ol",
                               tag="outl", bufs=10)
                for li in range(nl):
                    for m in range(M3):
                        sc = pf32_t[:, j, l0 + li, m:m + 1]
                        nc.scalar.mul(ol[:, li, m, :], y_half[:], sc)
                store_eng.dma_start(
                    odram[j * 128:(j + 1) * 128, l0:l0 + nl, :, :], ol[:])

            # ---- y2 head (prologue): borrows Pifo[0] banks 0-1 ----
            y2b = []
            for jj in range(NJ // 4):
                tgt4 = Pifo[0][:, jj, 0:512]
                for j4 in range(4):
                    j = jj * 4 + j4
                    tgt = Pifo[0][:, jj, j4 * 128:(j4 + 1) * 128]
                    nc.tensor.matmul(tgt, cnt_t[:, j * 128:(j + 1) * 128],
                                     fcw_t[:], start=True, stop=False)
                    nc.tensor.matmul(tgt, ones_t[:], fcb_t[:],
                                     start=False, stop=True)
                yb = pool.tile([128, 512], F16, name="y2b", tag="y2b",
                               bufs=NJ // 4)
                nc.scalar.activation(yb[:], tgt4, AF.Relu)
                y2b.append(yb)

            # y2 einsum job list: (engine, j, l0, nl)
            y2_jobs = []
            for j in range(NJ):
                if j < 5:
                    y2_jobs.append(('v', j, 0, 5))
                    y2_jobs.append(('v', j, 5, 5))
                elif j == 5:
                    y2_jobs.append(('v', j, 0, 5))
                    y2_jobs.append(('g', j, 5, 2))
                    y2_jobs.append(('g', j, 7, 3))
                else:
                    for l0, nl in ((0, 2), (2, 3), (5, 2), (7, 3)):
                        y2_jobs.append(('g', j, l0, nl))

            def y2_src(j):
                return y2b[j // 4][:, (j % 4) * 128:(j % 4) * 128 + 128]

            # torch gate order: 0=i, 1=f, 2=g(cell), 3=o.
            # Pifo slots 0,1,2 hold i,f,o (one sigmoid read); Pg holds g.
            GORD = (0, 1, 3, 2)          # emission order: sigmoid gates first

            def dst_for(cb, g):
                if g == 2:
                    return Pg[cb][:, 0:CW]
                return Pifo[cb][:, (2 if g == 3 else g), 0:CW]

            def emit_proj8(t, cb, start):
                cs = slice(cb * CW, (cb + 1) * CW)
                for g in GORD:
                    nc.tensor.matmul(
                        dst_for(cb, g),
                        wp8_t[:, :, g * 128:(g + 1) * 128],
                        x8_t[:, :, t, cs],
                        start=start, stop=False, perf_mode=DR)

            def emit_combo8(t, cb):
                # moving slot0 = h8(t-1) (plane t-1), slot1 = xtail(t)
                # (host stored xtail plane-shifted by -1)
                cs = slice(cb * CW, (cb + 1) * CW)
                mv = hx8[:, :, t - 1, cs]
                for g in GORD:
                    nc.tensor.matmul(
                        dst_for(cb, g),
                        wc8_t[:, :, g * 128:(g + 1) * 128],
                        mv, start=False, stop=True, perf_mode=DR)

            def emit_tail0(cb):
                # t=0: no h yet -> xtail-only regular fp8 matmul
                cs = slice(cb * CW, (cb + 1) * CW)
                mv = hx8[:, 1, TC - 1, cs]      # plane TC-1 = xtail[0]
                for g in GORD:
                    nc.tensor.matmul(
                        dst_for(cb, g),
                        wc8_t[:, 1, g * 128:(g + 1) * 128],
                        mv, start=False, stop=True)

            def emit_proj16(t, cb, start):
                # t is absolute; x16 tiles indexed t-TC
                cs = slice(cb * CW, (cb + 1) * CW)
                for g in GORD:
                    dst = dst_for(cb, g)
                    for k in range(3):
                        nc.tensor.matmul(
                            dst, w16_t[k][:, g * 128:(g + 1) * 128],
                            x16_t[k][:, t - TC, cs],
                            start=(start and k == 0), stop=False)

            def emit_rec16(cb, h_prev):
                for g in GORD:
                    nc.tensor.matmul(
                        dst_for(cb, g),
                        whh_t[:, g * 128:(g + 1) * 128],
                        h_prev[:], start=False, stop=True)

            def new_state(tag):
                return pool.tile([128, CW], F16, name=tag, tag=tag, bufs=2)

            h_prev = [None, None]     # fp16 h tiles (t >= TC-1)
            c_prev = [None, None]

            # ---- prologue: projections for t=0 ----
            emit_proj8(0, 0, start=True)
            emit_proj8(0, 1, start=True)
            emit_tail0(0)
            emit_tail0(1)

            # ---- recurrence ----
            for t in range(T):
                if 1 <= t < TC:
                    emit_combo8(t, 0)
                    emit_combo8(t, 1)
                elif t >= TC:
                    emit_rec16(0, h_prev[0])
                    emit_rec16(1, h_prev[1])

                gifo = [pool.tile([128, 3, CW], F16, name="gifo",
                                  tag=f"gifo{cb}", bufs=2)
                        for cb in range(NCB)]
                gg = [new_state(f"gg{cb}") for cb in range(NCB)]
                c_t = ([new_state(f"c{cb}") for cb in range(NCB)]
                       if t > 0 else [None, None])
                tc_t = [new_state(f"tc{cb}") for cb in range(NCB)]
                t1 = ([new_state(f"t1{cb}") for cb in range(NCB)]
                      if t > 0 else [None, None])
                t2 = [new_state(f"t2{cb}") for cb in range(NCB)]

                for cb in range(NCB):
                    nc.scalar.activation(gifo[cb][:], Pifo[cb][:, :, 0:CW],
                                         AF.Sigmoid)
                    nc.scalar.activation(gg[cb][:], Pg[cb][:, 0:CW], AF.Tanh)
                    if t > 0:
                        nc.vector.tensor_mul(t1[cb][:], gifo[cb][:, 1, :],
                                             c_prev[cb][:])
                    nc.vector.tensor_mul(t2[cb][:], gifo[cb][:, 0, :],
                                         gg[cb][:])
                    if t > 0:
                        nc.vector.tensor_add(c_t[cb][:], t1[cb][:],
                                             t2[cb][:])
                    else:
                        c_t[cb] = t2[cb]
                # tanh(c) + h after both chains' gate ACTs are queued
                h_t = [None, None]
                for cb in range(NCB):
                    nc.scalar.activation(tc_t[cb][:], c_t[cb][:], AF.Tanh)
                    cs = slice(cb * CW, (cb + 1) * CW)
                    if t < TC - 1:
                        # h8 -> hx8 slot0 plane t (read by combo at t+1)
                        nc.vector.tensor_mul(hx8[:, 0, t, cs],
                                             gifo[cb][:, 2, :], tc_t[cb][:])
                    else:
                        h_t[cb] = new_state(f"h{cb}")
                        nc.vector.tensor_mul(h_t[cb][:], gifo[cb][:, 2, :],
                                             tc_t[cb][:])

                # projections for t+1
                if t + 1 < TC:
                    emit_proj8(t + 1, 0, start=True)
                    emit_proj8(t + 1, 1, start=True)
                elif t + 1 < T:
                    emit_proj16(t + 1, 0, start=True)
                    emit_proj16(t + 1, 1, start=True)

                h_prev = h_t
                c_prev = c_t

                # y2 einsum jobs ride the recurrence (ACT-free)
                if t >= 2 and y2_jobs:
                    if t < T - 1:
                        take, nv, ng = [], 0, 0
                        for job in y2_jobs:
                            if job[0] == 'v' and nv < 1:
                                take.append(job); nv += 1
                            elif job[0] == 'g' and ng < 1:
                                take.append(job); ng += 1
                        for job in take:
                            y2_jobs.remove(job)
                            _, j, l0, nl = job
                            eng = nc.vector if job[0] == 'v' else nc.gpsimd
                            emit_einsum_bcast(j, y2_src(j), oy2, l0, nl,
                                              eng, nc.gpsimd)
                    else:
                        for job in y2_jobs:
                            _, j, l0, nl = job
                            eng = nc.vector if job[0] == 'v' else nc.gpsimd
                            emit_einsum_bcast(j, y2_src(j), oy2, l0, nl,
                                              eng, nc.gpsimd)
                        y2_jobs = []

            # ---- tail: y1 half ----
            NJH = NJ // 2
            y1b = []
            for j in range(NJ):
                y1 = pool.tile([128, 128], F16, name="y1b", tag="y1b",
                               bufs=NJ)
                src = h_prev[j // NJH][:, (j % NJH) * 128:(j % NJH) * 128 + 128]
                eng = nc.sync if j % 2 == 0 else nc.scalar
                eng.dma_start_transpose(y1[:], src)
                y1b.append(y1)
            # 16 jobs: DVE 11, GpSimd 4, ACT 1
            jobs = [(j, l0) for j in range(NJ) for l0 in range(0, L, 5)]
            for idx, (j, l0) in enumerate(jobs):
                st = nc.sync if idx % 2 == 0 else nc.scalar
                r = idx % 8
                if r in (2, 6):
                    emit_einsum_bcast(j, y1b[j], oy1, l0, 5, nc.gpsimd, st)
                elif r == 4 and idx < 8:
                    emit_einsum_act(j, y1b[j], oy1, l0, 5, st)
                else:
                    emit_einsum_bcast(j, y1b[j], oy1, l0, 5, nc.vector, st)

    nc.compile()
    return nc


def prep_in_maps(inputs, n_cores: int, b_loc: int):
    history = np.asarray(inputs["history"], np.float32)
    cluster = np.asarray(inputs["cluster_num"], np.float32)
    pref = np.asarray(inputs["pref"], np.float32)
    w_ih = np.asarray(inputs["W_ih"], np.float32)
    w_hh = np.asarray(inputs["W_hh"], np.float32)
    b_ih = np.asarray(inputs["b_ih"], np.float32)
    b_hh = np.asarray(inputs["b_hh"], np.float32)
    fc_w = np.asarray(inputs["fc_w"], np.float32)
    fc_b = np.asarray(inputs["fc_b"], np.float32)

    NJ = b_loc // 128
    T16 = T - TC
    bias = (b_ih + b_hh).reshape(1, 4 * H)
    w_ihT = w_ih.T                                   # [D, 4H]

    # fp8 weights
    wp8 = np.empty((128, 2, 4 * H), np.float32)
    wp8[:, 0] = w_ihT[0:128]
    wp8[:, 1] = w_ihT[128:256]
    wc8 = np.zeros((128, 2, 4 * H), np.float32)
    wc8[:, 0] = w_hh.T
    wc8[0:84, 1] = w_ihT[256:340]
    wc8[84, 1] = bias[0]
    # fp16 weights (with bias row at DP-1)
    w16 = np.concatenate([w_ihT, bias], axis=0)      # [341, 512]

    shared = {
        "wp8": wp8.astype(E4NP),
        "wc8": wc8.astype(E4NP),
        "w_hh_t": np.ascontiguousarray(w_hh.T.astype(np.float16)),
        "fc_w_t": np.ascontiguousarray(fc_w.T.astype(np.float16)),
        "fc_b_row": np.ascontiguousarray(fc_b.reshape(1, H).astype(np.float16)),
        "ones_row": np.ones((1, 128), np.float16),
    }
    for k, (c0, c1) in enumerate(DCH):
        shared[f"w16_{k}"] = np.ascontiguousarray(w16[c0:c1].astype(np.float16))

    in_maps = []
    for c in range(n_cores):
        r0, r1 = c * b_loc, (c + 1) * b_loc
        histT = np.ascontiguousarray(
            history[r0:r1].reshape(b_loc, T, D).transpose(2, 1, 0))  # [D,T,b]
        # fp8 proj rows 0:256 as 2 slots, steps 0..TC-1
        x8 = np.empty((128, 2, TC, b_loc), np.float32)
        x8[:, 0] = histT[0:128, :TC]
        x8[:, 1] = histT[128:256, :TC]
        # fp8 xtail, plane-shifted: plane p = xtail[t=p+1], plane TC-1 = t=0
        x8t = np.zeros((128, TC, b_loc), np.float32)
        tsrc = np.roll(np.arange(TC), -1)            # plane p <- t=(p+1)%TC
        x8t[0:84] = histT[256:340][:, tsrc]
        x8t[84] = 1.0
        # fp16 steps TC..T-1 with bias-ones row
        x16full = np.empty((DP, T16, b_loc), np.float16)
        x16full[:D] = histT[:, TC:].astype(np.float16)
        x16full[D] = 1.0

        pref16 = pref[r0:r1].reshape(NJ, 128, L, M3).astype(np.float16)
        pg = np.ascontiguousarray(pref16.transpose(1, 0, 2, 3))
        m = {
            "x8": x8.astype(E4NP),
            "x8t": x8t.astype(E4NP),
            "cn_t": np.ascontiguousarray(cluster[r0:r1].T.astype(np.float16)),
            "pref_g": pg,
            "pref_g32": pg.astype(np.float32),
            **shared,
        }
        for k, (c0, c1) in enumerate(DCH):
            m[f"x16_{k}"] = np.ascontiguousarray(x16full[c0:c1])
        in_maps.append(m)
    return in_maps


def run(inputs, n_cores: int = N_CORES, trace: bool = False):
    B = np.asarray(inputs["history"]).shape[0]
    b_loc = B // n_cores
    nc = build_program(b_loc)
    in_maps = prep_in_maps(inputs, n_cores, b_loc)
    res = bass_utils.run_bass_kernel_spmd(
        nc, in_maps, core_ids=list(range(n_cores)), trace=trace)
    outs = []
    for c in range(n_cores):
        y1 = res.results[c]["out_y1"].astype(np.float32)
        y2 = res.results[c]["out_y2"].astype(np.float32)
        o = np.concatenate([y1, y2], axis=3)         # [b, L, M3, 256]
        outs.append(o.transpose(0, 1, 3, 2).reshape(b_loc, L, 256 * M3))
    return np.concatenate(outs, axis=0), res


def kernel(**inputs) -> np.ndarray:
    out, _ = run(inputs, N_CORES)
    return out


# revision 6
# speedup vs baseline: 1.1446x; 1.1446x over previous
"""Trainium2 Bass kernel for nn_BackBone (LSTM backbone + fc + outer-product head).

Data-parallel over batch across 8 NeuronCores. Per core (b_loc rows), v6:
  - history transposed + cast to fp16 on the HOST: xt[D+1, T, B] with a
    constant-1.0 feature row appended so the gate bias rides the projection
    matmul. All matmuls fp16 (fp8 DoubleRow measured 424ns per 512-col MM =
    exactly two fp16 matmuls; no streaming win, so fp16 keeps accuracy).
  - two 512-col batch chains, per-READER gate PSUMs (Pif merged sigmoid(i,f),
    Pg tanh, Po sigmoid) so proj(t+1) matmuls only wait on their own gate's
    activation read -> zero-stall PE steady state at ~213ns/MM.
  - head einsum: broadcast tensor_tensor with PAIR-DUPLICATED pref
    (pf2[...,2]); the step-1 inner pair unlocks DVE 2x_1P mode: 1.15us per
    [128,5,3,128] job vs 2.15us classic. y2 jobs ride the recurrence on
    DVE + small GpSimd bites (no ACT einsum while ACT paces the LSTM).
  - y1 tail: dma_start_transpose h on sync+scalar queues, einsum jobs split
    DVE(12)/ACT(2)/GpSimd(2), stores on sync+scalar; y2 stores on gpsimd.
"""
import numpy as np

import concourse.bacc as bacc
import concourse.mybir as mybir
import concourse.tile as tile
from concourse import bass_utils

F32 = mybir.dt.float32
F16 = mybir.dt.float16
AF = mybir.ActivationFunctionType

T = 20
D = 340
DP = D + 1               # +1 constant feature row carrying the gate bias
H = 128
E = 32
L = 10
M3 = 3
DCH = [(0, 128), (128, 256), (256, DP)]   # contraction chunks of DP
N_CORES = 8


def build_program(b_loc: int):
    assert b_loc % 256 == 0
    NJ = b_loc // 128
    CW = b_loc // 2               # chain width (<= 512)
    assert CW <= 512
    NCB = 2
    if T == 20:
        TGR = [(0, 1), (1, 2)] + [(t, t + 2) for t in range(2, 14, 2)] \
            + [(14, 17), (17, 20)]
    else:
        TGR = [(0, T)]

    nc = bacc.Bacc("TRN2", target_bir_lowering=False, debug=False)
    xt_d = nc.dram_tensor("xt", (DP, T, b_loc), F16, kind="ExternalInput").ap()
    cnt_d = nc.dram_tensor("cn_t", (E, b_loc), F16, kind="ExternalInput").ap()
    pref2_d = nc.dram_tensor("pref2", (128, NJ, L, M3, 2), F16,
                             kind="ExternalInput").ap()
    pref32_d = nc.dram_tensor("pref_g32", (128, NJ, L, M3), F32,
                              kind="ExternalInput").ap()
    wih_d = nc.dram_tensor("w_ih4", (DP, 4 * H), F16, kind="ExternalInput").ap()
    whh_d = nc.dram_tensor("w_hh_t", (H, 4 * H), F16, kind="ExternalInput").ap()
    fcw_d = nc.dram_tensor("fc_w_t", (E, H), F16, kind="ExternalInput").ap()
    fcb_d = nc.dram_tensor("fc_b_row", (1, H), F16, kind="ExternalInput").ap()
    ones_d = nc.dram_tensor("ones_row", (1, 128), F16, kind="ExternalInput").ap()
    oy1 = nc.dram_tensor("out_y1", (b_loc, L, M3, 128), F16,
                         kind="ExternalOutput").ap()
    oy2 = nc.dram_tensor("out_y2", (b_loc, L, M3, 128), F16,
                         kind="ExternalOutput").ap()

    with tile.TileContext(nc) as tc:
        with tc.tile_pool(name="wpool", bufs=1) as wpool, \
             tc.tile_pool(name="main", bufs=1) as pool, \
             tc.tile_pool(name="psum", bufs=1, space="PSUM") as pspool:

            # ---- weights / constants ----
            wih_t = []
            for k, (c0, c1) in enumerate(DCH):
                wt_ = wpool.tile([c1 - c0, 4 * H], F16, name=f"wih{k}")
                nc.sync.dma_start(wt_[:], wih_d[c0:c1, :])
                wih_t.append(wt_)
            whh_t = wpool.tile([H, 4 * H], F16, name="whh_t")
            nc.sync.dma_start(whh_t[:], whh_d)
            cnt_t = wpool.tile([E, b_loc], F16, name="cnt_t")
            nc.gpsimd.dma_start(cnt_t[:], cnt_d)
            fcw_t = wpool.tile([E, H], F16, name="fcw_t")
            nc.gpsimd.dma_start(fcw_t[:], fcw_d)
            fcb_t = wpool.tile([1, H], F16, name="fcb_t")
            nc.gpsimd.dma_start(fcb_t[:], fcb_d)
            ones_t = wpool.tile([1, 128], F16, name="ones_t")
            nc.gpsimd.dma_start(ones_t[:], ones_d)
            pf2_t = wpool.tile([128, NJ, L, M3, 2], F16, name="pf2_t")
            nc.gpsimd.dma_start(pf2_t[:], pref2_d)
            pf32_t = wpool.tile([128, NJ, L, M3], F32, name="pf32_t")
            nc.gpsimd.dma_start(pf32_t[:], pref32_d)

            # ---- persistent fp16 xT tiles, loaded in t-groups ----
            xt_tiles = []
            for k, (c0, c1) in enumerate(DCH):
                xt_tiles.append(
                    pool.tile([c1 - c0, T, b_loc], F16, name=f"xt{k}",
                              tag=f"xt{k}"))
            for (t0, t1) in TGR:
                for k, (c0, c1) in enumerate(DCH):
                    nc.sync.dma_start(xt_tiles[k][:, t0:t1, :],
                                      xt_d[c0:c1, t0:t1, :])

            # ---- PSUM per chain, split per ACT reader ----
            Pif, Pg, Po = [], [], []
            for cb in range(NCB):
                Pif.append(pspool.tile([128, 2, 512], F32, name=f"pif{cb}",
                                       tag=f"pif{cb}"))
                Pg.append(pspool.tile([128, 512], F32, name=f"pg{cb}",
                                      tag=f"pg{cb}"))
                Po.append(pspool.tile([128, 512], F32, name=f"po{cb}",
                                      tag=f"po{cb}"))

            def emit_einsum_p2(j, y_half, odram, l0, nl, engine, store_eng):
                """pair-duplicated pref broadcast mul: DVE 2x_1P mode."""
                ol = pool.tile([128, nl, M3, 128], F16, name="ol",
                               tag="outl", bufs=10)
                y_b = y_half[:, None, None, :].rearrange(
                    "p a b (n t) -> p a b n t", t=2).broadcast_to(
                    [128, nl, M3, 64, 2])
                p_b = pf2_t[:, j, l0:l0 + nl, :, None, :].broadcast_to(
                    [128, nl, M3, 64, 2])
                engine.tensor_mul(
                    ol[:].rearrange("p a b (n t) -> p a b n t", t=2),
                    y_b, p_b)
                store_eng.dma_start(
                    odram[j * 128:(j + 1) * 128, l0:l0 + nl, :, :], ol[:])

            def emit_einsum_act(j, y_half, odram, l0, nl, store_eng):
                ol = pool.tile([128, nl, M3, 128], F16, name="ol",
                               tag="outl", bufs=10)
                for li in range(nl):
                    for m in range(M3):
                        sc = pf32_t[:, j, l0 + li, m:m + 1]
                        nc.scalar.mul(ol[:, li, m, :], y_half[:], sc)
                store_eng.dma_start(
                    odram[j * 128:(j + 1) * 128, l0:l0 + nl, :, :], ol[:])

            # ---- y2 head (prologue): borrows Pif[0] banks ----
            y2b = []
            for jj in range(NJ // 4):
                tgt4 = Pif[0][:, jj, 0:512]
                for j4 in range(4):
                    j = jj * 4 + j4
                    tgt = Pif[0][:, jj, j4 * 128:(j4 + 1) * 128]
                    nc.tensor.matmul(tgt, cnt_t[:, j * 128:(j + 1) * 128],
                                     fcw_t[:], start=True, stop=False)
                    nc.tensor.matmul(tgt, ones_t[:], fcb_t[:],
                                     start=False, stop=True)
                yb = pool.tile([128, 512], F16, name="y2b", tag="y2b",
                               bufs=max(1, NJ // 4))
                nc.scalar.activation(yb[:], tgt4, AF.Relu)
                y2b.append(yb)

            def y2_src(j):
                return y2b[j // 4][:, (j % 4) * 128:(j % 4) * 128 + 128]

            # y2 einsum jobs: ('v', j, l0, nl) on DVE, ('g', ...) on GpSimd
            y2_jobs = []
            for j in range(NJ):
                if j < 6:
                    y2_jobs.append(('v', j, 0, 5))
                    y2_jobs.append(('v', j, 5, 5))
                else:
                    for l0, nl in ((0, 2), (2, 3), (5, 2), (7, 3)):
                        y2_jobs.append(('g', j, l0, nl))

            def emit_proj(t, cb, stop):
                cs = slice(cb * CW, (cb + 1) * CW)

                def mm(dst, g, k):
                    nc.tensor.matmul(
                        dst, wih_t[k][:, g * 128:(g + 1) * 128],
                        xt_tiles[k][:, t, cs],
                        start=(k == 0), stop=(stop and k == 2))
                for k in range(3):              # i, f pairs first
                    mm(Pif[cb][:, 0, 0:CW], 0, k)
                    mm(Pif[cb][:, 1, 0:CW], 1, k)
                for k in range(3):              # then g (cell)
                    mm(Pg[cb][:, 0:CW], 2, k)
                for k in range(3):              # then o
                    mm(Po[cb][:, 0:CW], 3, k)

            def emit_rec(cb, h_prev):
                for g, dst in ((0, Pif[cb][:, 0, 0:CW]),
                               (1, Pif[cb][:, 1, 0:CW]),
                               (2, Pg[cb][:, 0:CW]),
                               (3, Po[cb][:, 0:CW])):
                    nc.tensor.matmul(dst, whh_t[:, g * 128:(g + 1) * 128],
                                     h_prev[:], start=False, stop=True)

            def new_state(tag):
                return pool.tile([128, CW], F16, name=tag, tag=tag, bufs=2)

            h_prev = [None, None]
            c_prev = [None, None]

            # ---- prologue projections for t=0 ----
            emit_proj(0, 0, stop=True)
            emit_proj(0, 1, stop=True)

            # ---- recurrence ----
            for t in range(T):
                if t > 0:
                    emit_rec(0, h_prev[0])
                    emit_rec(1, h_prev[1])

                gif = [pool.tile([128, 2, CW], F16, name="gif",
                                 tag=f"gif{cb}", bufs=2) for cb in range(NCB)]
                gg = [new_state(f"gg{cb}") for cb in range(NCB)]
                go = [new_state(f"go{cb}") for cb in range(NCB)]
                c_t = ([new_state(f"c{cb}") for cb in range(NCB)]
                       if t > 0 else [None, None])
                tc_t = [new_state(f"tc{cb}") for cb in range(NCB)]
                h_t = [new_state(f"h{cb}") for cb in range(NCB)]
                t1 = ([new_state(f"t1{cb}") for cb in range(NCB)]
                      if t > 0 else [None, None])
                t2 = [new_state(f"t2{cb}") for cb in range(NCB)]

                for cb in range(NCB):
                    nc.scalar.activation(gif[cb][:], Pif[cb][:, :, 0:CW],
                                         AF.Sigmoid)
                    nc.scalar.activation(gg[cb][:], Pg[cb][:, 0:CW], AF.Tanh)
                    nc.scalar.activation(go[cb][:], Po[cb][:, 0:CW],
                                         AF.Sigmoid)
                    if t > 0:
                        nc.vector.tensor_mul(t1[cb][:], gif[cb][:, 1, :],
                                             c_prev[cb][:])
                    nc.vector.tensor_mul(t2[cb][:], gif[cb][:, 0, :],
                                         gg[cb][:])
                    if t > 0:
                        nc.vector.tensor_add(c_t[cb][:], t1[cb][:],
                                             t2[cb][:])
                    else:
                        c_t[cb] = t2[cb]
                # tanh(c) + h after both chains' gate ACTs are queued
                for cb in range(NCB):
                    nc.scalar.activation(tc_t[cb][:], c_t[cb][:], AF.Tanh)
                    nc.vector.tensor_mul(h_t[cb][:], go[cb][:], tc_t[cb][:])

                # PE: projections for t+1
                if t + 1 < T:
                    emit_proj(t + 1, 0, stop=False)
                    emit_proj(t + 1, 1, stop=False)

                h_prev = h_t
                c_prev = c_t

                # y2 einsum through the recurrence: 1 DVE job + 1 gpsimd
                # bite per step (small bites: gpsimd shares an SBUF port
                # with DVE)
                if t >= 2 and y2_jobs:
                    if t < T - 1:
                        take, nv, ng = [], 0, 0
                        for job in y2_jobs:
                            if job[0] == 'v' and nv < 1:
                                take.append(job); nv += 1
                            elif job[0] == 'g' and ng < 1:
                                take.append(job); ng += 1
                        for job in take:
                            y2_jobs.remove(job)
                    else:
                        take, y2_jobs = list(y2_jobs), []
                    for kind, j, l0, nl in take:
                        eng = nc.vector if kind == 'v' else nc.gpsimd
                        emit_einsum_p2(j, y2_src(j), oy2, l0, nl,
                                       eng, nc.gpsimd)

            # ---- tail: y1 half ----
            NJH = NJ // 2
            y1b = []
            for j in range(NJ):
                y1 = pool.tile([128, 128], F16, name="y1b", tag="y1b",
                               bufs=NJ)
                src = h_prev[j // NJH][:, (j % NJH) * 128:(j % NJH) * 128 + 128]
                eng = nc.sync if j % 2 == 0 else nc.scalar
                eng.dma_start_transpose(y1[:], src)
                y1b.append(y1)
            # 16 jobs: DVE 12, ACT 2, GpSimd 2
            jobs = [(j, l0) for j in range(NJ) for l0 in range(0, L, 5)]
            for idx, (j, l0) in enumerate(jobs):
                st = nc.sync if idx % 2 == 0 else nc.scalar
                r = idx % 8
                if r == 3:
                    emit_einsum_act(j, y1b[j], oy1, l0, 5, st)
                elif r == 6:
                    emit_einsum_p2(j, y1b[j], oy1, l0, 5, nc.gpsimd, st)
                else:
                    emit_einsum_p2(j, y1b[j], oy1, l0, 5, nc.vector, st)

    nc.compile()
    return nc


def prep_in_maps(inputs, n_cores: int, b_loc: int):
    history = np.asarray(inputs["history"], np.float32)
    cluster = np.asarray(inputs["cluster_num"], np.float32)
    pref = np.asarray(inputs["pref"], np.float32)
    w_ih = np.asarray(inputs["W_ih"], np.float32)
    w_hh = np.asarray(inputs["W_hh"], np.float32)
    b_ih = np.asarray(inputs["b_ih"], np.float32)
    b_hh = np.asarray(inputs["b_hh"], np.float32)
    fc_w = np.asarray(inputs["fc_w"], np.float32)
    fc_b = np.asarray(inputs["fc_b"], np.float32)

    NJ = b_loc // 128
    w_ih4 = np.concatenate(
        [w_ih.T, (b_ih + b_hh).reshape(1, 4 * H)], axis=0)  # [341, 512]
    shared = {
        "w_ih4": np.ascontiguousarray(w_ih4.astype(np.float16)),
        "w_hh_t": np.ascontiguousarray(w_hh.T.astype(np.float16)),
        "fc_w_t": np.ascontiguousarray(fc_w.T.astype(np.float16)),
        "fc_b_row": np.ascontiguousarray(fc_b.reshape(1, H).astype(np.float16)),
        "ones_row": np.ones((1, 128), np.float16),
    }
    in_maps = []
    for c in range(n_cores):
        r0, r1 = c * b_loc, (c + 1) * b_loc
        hist16 = history[r0:r1].reshape(b_loc, T, D).astype(np.float16)
        xt = np.empty((DP, T, b_loc), np.float16)
        xt[:D] = hist16.transpose(2, 1, 0)
        xt[D] = 1.0
        pref16 = pref[r0:r1].reshape(NJ, 128, L, M3).astype(np.float16)
        pg = np.ascontiguousarray(pref16.transpose(1, 0, 2, 3))
        in_maps.append({
            "xt": xt,
            "cn_t": np.ascontiguousarray(
                cluster[r0:r1].T.astype(np.float16)),
            "pref2": np.ascontiguousarray(
                np.repeat(pg[..., None], 2, axis=-1)),
            "pref_g32": pg.astype(np.float32),
            **shared,
        })
    return in_maps


def run(inputs, n_cores: int = N_CORES, trace: bool = False):
    B = np.asarray(inputs["history"]).shape[0]
    b_loc = B // n_cores
    nc = build_program(b_loc)
    in_maps = prep_in_maps(inputs, n_cores, b_loc)
    res = bass_utils.run_bass_kernel_spmd(
        nc, in_maps, core_ids=list(range(n_cores)), trace=trace)
    outs = []
    for c in range(n_cores):
        y1 = res.results[c]["out_y1"].astype(np.float32)
        y2 = res.results[c]["out_y2"].astype(np.float32)
        o = np.concatenate([y1, y2], axis=3)         # [b, L, M3, 256]
        outs.append(o.transpose(0, 1, 3, 2).reshape(b_loc, L, 256 * M3))
    return np.concatenate(outs, axis=0), res


def kernel(**inputs) -> np.ndarray:
    out, _ = run(inputs, N_CORES)
    return out
